# revision 1
# baseline (speedup 1.0000x reference)
"""Trainium2 Bass kernel for nn_DomainAdaption (conv-conv-MoE-gated-residual).

Data-parallel over batch: 16 samples -> 8 NeuronCores, 2 samples/core.
Per sample on-device (conv3x3 = 9 shifted accumulated matmuls over a
host-padded [C,130,130] map, channels on partitions, 4-row x 128-col chunks):
  h1 = prelu(conv3x3(x, w1) + b1c)     float32r matmuls, ScalarE Prelu epilogue
  h2 = conv3x3(h1, w2) + b2c           bf16 matmuls, ScalarE epilogue + accum_out
  x1 = mean(h2, spatial)               accum_out columns + tensor_reduce
  a  = relu(W1[e] @ x1 + b1)           expert weights gathered on host
  g  = sigmoid(W2[e] @ a + b2)
  out = prelu(h2 * g + x)              sample 0: VectorE STT fusions (overlaps
                                       sample 1's convs); sample 1 (the tail):
                                       diag(g) & identity matmuls on the idle
                                       TensorE + ScalarE Prelu off PSUM
"""
import sys

if "/opt/trn_rl_repo" not in sys.path:
    sys.path.insert(0, "/opt/trn_rl_repo")

import numpy as np
import ml_dtypes

N, C, H, W = 16, 128, 128, 128
CH = 32
NCORES = 8
SPC = N // NCORES          # samples per core
HP = H + 2                 # padded rows/cols
NCHUNK = H // 4            # 32 chunks of 4 rows (512 spatial positions)
BF = ml_dtypes.bfloat16


def _build(prelu1: float, prelu2: float, conv_bias: bool):
    import concourse.mybir as mybir
    import concourse.tile as tile
    from concourse import bacc

    F32 = mybir.dt.float32
    F32R = mybir.dt.float32r
    BF16 = mybir.dt.bfloat16
    AF = mybir.ActivationFunctionType
    ALU = mybir.AluOpType

    nc = bacc.Bacc("TRN2", target_bir_lowering=False, debug=False,
                   num_devices=NCORES)

    # x arrives host-padded: [SPC, C, 130, 130] with the zero ring baked in
    x_d = nc.dram_tensor("x", [SPC, C, HP, HP], F32R, kind="ExternalInput").ap()
    cw1_d = nc.dram_tensor("cw1", [C, 9, C], F32R, kind="ExternalInput").ap()
    cw2_d = nc.dram_tensor("cw2", [C, 9, C], BF16, kind="ExternalInput").ap()
    c1b_d = nc.dram_tensor("c1b", [C, 1], F32, kind="ExternalInput").ap()
    c2b_d = nc.dram_tensor("c2b", [C, 1], F32, kind="ExternalInput").ap()
    w1t_d = nc.dram_tensor("w1t", [SPC, C, CH], F32, kind="ExternalInput").ap()
    b1_d = nc.dram_tensor("b1", [SPC, CH, 1], F32, kind="ExternalInput").ap()
    w2t_d = nc.dram_tensor("w2t", [SPC, CH, C], F32, kind="ExternalInput").ap()
    b2_d = nc.dram_tensor("b2", [SPC, C, 1], F32, kind="ExternalInput").ap()
    ident_d = nc.dram_tensor("ident", [C, C], F32R, kind="ExternalInput").ap()
    y_d = nc.dram_tensor("y", [SPC, C, H, W], F32, kind="ExternalOutput").ap()

    with tile.TileContext(nc) as tc, (
        tc.tile_pool(name="wp", bufs=1)) as wp, (
        tc.tile_pool(name="xp", bufs=1)) as xp, (
        tc.tile_pool(name="h1p", bufs=1)) as h1p, (
        tc.tile_pool(name="h2p", bufs=2)) as h2p, (
        tc.tile_pool(name="adw", bufs=2)) as adw, (
        tc.tile_pool(name="vec", bufs=2)) as vec, (
        tc.tile_pool(name="xrp", bufs=1)) as xrp, (
        tc.tile_pool(name="otp", bufs=3)) as otp, (
        tc.tile_pool(name="psc", bufs=6, space="PSUM")) as psc, (
        tc.tile_pool(name="psv", bufs=1, space="PSUM")) as psv:

        cw1_t = wp.tile([C, 9, C], F32R)
        c1b_t = wp.tile([C, 1], F32)
        cw2_t = wp.tile([C, 9, C], BF16)
        c2b_t = wp.tile([C, 1], F32)
        ident_t = wp.tile([C, C], F32R)

        # x in 4 row-band tiles (34 padded rows each, 2-row overlap) so conv1
        # chunks depend only on the band they read. Band k = padded rows
        # 32k .. 32k+33; full 130-col width with host-baked zero ring.
        xb = [xp.tile([C, 34, HP], F32R, name=f"xb{k}") for k in range(4)]

        h1_pad = h1p.tile([C, HP, HP], BF16)
        nc.vector.memset(h1_pad[:, 0, :], 0)
        nc.vector.memset(h1_pad[:, HP - 1, :], 0)
        nc.vector.memset(h1_pad[:, 1:HP - 1, 0], 0)
        nc.vector.memset(h1_pad[:, 1:HP - 1, HP - 1], 0)

        for s in range(SPC):
            # conv1 (float32r) -> prelu -> h1_pad; banded x loads so chunk c
            # only waits on band c//8. Weights stream in behind band 0 so the
            # first matmul isn't queued behind them on the serial DMA path.
            for k in range(4):
                if s == 0 and k == 0:
                    nc.scalar.dma_start(cw1_t[:], cw1_d)
                    nc.scalar.dma_start(c1b_t[:], c1b_d)
                    nc.sync.dma_start(xb[k][:, 0:7], x_d[s, :, 0:7, :])
                    nc.sync.dma_start(xb[k][:, 7:18], x_d[s, :, 7:18, :])
                else:
                    nc.sync.dma_start(xb[k][:, 0:18],
                                      x_d[s, :, 32 * k:32 * k + 18, :])
                nc.sync.dma_start(xb[k][:, 18:34],
                                  x_d[s, :, 32 * k + 18:32 * k + 34, :])
                if s == 0 and k == 1:
                    nc.sync.dma_start(cw2_t[:], cw2_d)
                    nc.sync.dma_start(c2b_t[:], c2b_d)
                    nc.sync.dma_start(ident_t[:], ident_d)
                for c in range(8 * k, 8 * k + 8):
                    pch = psc.tile([C, 4, W], F32)
                    lr = 4 * (c - 8 * k)
                    for t in range(9):
                        dy, dx = t // 3, t % 3
                        nc.tensor.matmul(
                            pch[:], cw1_t[:, t, :],
                            xb[k][:, lr + dy:lr + dy + 4, dx:dx + W],
                            start=(t == 0), stop=(t == 8))
                    nc.scalar.activation(
                        h1_pad[:, 4 * c + 1:4 * c + 5, 1:W + 1], pch[:],
                        AF.Prelu, bias=(c1b_t[:] if conv_bias else 0.0),
                        alpha=prelu1)

            # conv2 (bf16) -> h2 + pooling partials (epilogue on ScalarE so
            # VectorE stays free for the previous sample's gated residual)
            h2 = h2p.tile([C, NCHUNK // 2, 8, W], BF16)
            stats = h2p.tile([C, NCHUNK], F32)
            for c in range(NCHUNK):
                pch = psc.tile([C, 4, W], F32)
                for t in range(9):
                    dy, dx = t // 3, t % 3
                    nc.tensor.matmul(
                        pch[:], cw2_t[:, t, :],
                        h1_pad[:, 4 * c + dy:4 * c + dy + 4, dx:dx + W],
                        start=(t == 0), stop=(t == 8))
                nc.scalar.activation(
                    h2[:, c // 2, (c % 2) * 4:(c % 2) * 4 + 4, :], pch[:],
                    AF.Identity, bias=(c2b_t[:] if conv_bias else 0.0),
                    accum_out=stats[:, c:c + 1])

            # per-sample expert (host-gathered) adapter weights
            w1t_t = adw.tile([C, CH], F32)
            nc.sync.dma_start(w1t_t[:], w1t_d[s])
            b1_t = adw.tile([CH, 1], F32)
            nc.sync.dma_start(b1_t[:], b1_d[s])
            w2t_t = adw.tile([CH, C], F32)
            nc.sync.dma_start(w2t_t[:], w2t_d[s])
            b2_t = adw.tile([C, 1], F32)
            nc.sync.dma_start(b2_t[:], b2_d[s])

            # global mean -> adapter MLP -> sigmoid gate
            x1 = vec.tile([C, 1], F32)
            nc.vector.tensor_reduce(x1[:], stats[:], axis=mybir.AxisListType.X,
                                    op=ALU.add)
            psa = psv.tile([CH, 1], F32)
            nc.tensor.matmul(psa[:], w1t_t[:], x1[:], start=True, stop=True)
            a_t = vec.tile([CH, 1], F32)
            nc.vector.tensor_scalar(a_t[:], psa[:], b1_t[:], 0.0,
                                    ALU.add, ALU.max)
            psg = psv.tile([C, 1], F32)
            nc.tensor.matmul(psg[:], w2t_t[:], a_t[:], start=True, stop=True)
            gate = vec.tile([C, 1], F32)
            nc.scalar.activation(gate[:], psg[:], AF.Sigmoid, bias=b2_t[:])

            # out = prelu(h2 * gate + x)
            if s == SPC - 1:
                # Tail sample: PE is idle now, so gate via a diagonal matmul
                # accumulated with identity @ x (x read from the resident f32r
                # bands), then Prelu straight off PSUM on ScalarE.
                diag_t = vec.tile([C, C], BF16)
                nc.vector.tensor_scalar_mul(
                    diag_t[:], ident_t[:].bitcast(F32), gate[:])
                for c in range(NCHUNK):
                    pch = psc.tile([C, 4, W], F32)
                    nc.tensor.matmul(
                        pch[:], diag_t[:],
                        h2[:, c // 2, (c % 2) * 4:(c % 2) * 4 + 4, :],
                        start=True, stop=False)
                    nc.tensor.matmul(
                        pch[:], ident_t[:],
                        xb[c // 8][:, 4 * (c % 8) + 1:4 * (c % 8) + 5,
                                   1:W + 1],
                        start=False, stop=True)
                    o_t = otp.tile([C, 4, W], F32, bufs=10, tag="ot")
                    nc.scalar.activation(o_t[:], pch[:], AF.Prelu,
                                         alpha=prelu2)
                    # alternate store-issue queues so no sequencer paces the
                    # tail (ScalarE only runs the 570ns Prelus)
                    if c % 2 == 0:
                        nc.sync.dma_start(y_d[s, :, 4 * c:4 * c + 4, :],
                                          o_t[:])
                    else:
                        nc.gpsimd.dma_start(y_d[s, :, 4 * c:4 * c + 4, :],
                                            o_t[:])
            else:
                # Overlapped sample: gated residual on VectorE while PE runs
                # the next sample's convs; residual re-streamed from HBM.
                for b in range(NCHUNK // 2):
                    xr = xrp.tile([C, 8, HP], F32)
                    nc.gpsimd.dma_start(
                        xr[:], x_d[s, :, 8 * b + 1:8 * b + 9, :].bitcast(F32))
                    for j in range(2):
                        c = 2 * b + j
                        t_t = otp.tile([C, 4, W], F32, bufs=10, tag="ot")
                        nc.vector.scalar_tensor_tensor(
                            t_t[:], h2[:, b, 4 * j:4 * j + 4, :], gate[:],
                            xr[:, 4 * j:4 * j + 4, 1:W + 1],
                            op0=ALU.mult, op1=ALU.add)
                        o_t = otp.tile([C, 4, W], F32, bufs=10, tag="ot")
                        # prelu(t) = max(a*t, t), valid for 0<=a<=1; on DVE
                        # so ScalarE only runs conv2 epilogues during the
                        # next sample's conv2 phase
                        if 0.0 <= prelu2 <= 1.0:
                            nc.vector.scalar_tensor_tensor(
                                o_t[:], t_t[:], prelu2, t_t[:],
                                op0=ALU.mult, op1=ALU.max)
                        else:
                            nc.scalar.activation(o_t[:], t_t[:], AF.Prelu,
                                                 alpha=prelu2)
                        nc.sync.dma_start(y_d[s, :, 4 * c:4 * c + 4, :],
                                          o_t[:])

    nc.compile()
    return nc


_CACHE = {}


def _get_program(prelu1, prelu2, conv_bias):
    key = (float(prelu1), float(prelu2), bool(conv_bias))
    if key not in _CACHE:
        _CACHE[key] = _build(*key)
    return _CACHE[key]


def _prep(x, intensity, conv1_w, conv1_b, prelu1, conv2_w, conv2_b,
          aW1, ab1, aW2, ab2, prelu2):
    x = np.asarray(x, np.float32)
    idx = np.asarray(intensity).astype(np.int64) - 1
    conv1_w = np.asarray(conv1_w, np.float32)
    conv1_b = np.asarray(conv1_b, np.float32)
    conv2_w = np.asarray(conv2_w, np.float32)
    conv2_b = np.asarray(conv2_b, np.float32)
    aW1 = np.asarray(aW1, np.float32)
    ab1 = np.asarray(ab1, np.float32)
    aW2 = np.asarray(aW2, np.float32)
    ab2 = np.asarray(ab2, np.float32)

    # [Co,Ci,ky,kx] -> [Ci, tap, Co] so lhsT slices are [K=Ci, M=Co]
    cw1 = np.ascontiguousarray(conv1_w.transpose(1, 2, 3, 0).reshape(C, 9, C))
    cw2 = np.ascontiguousarray(
        conv2_w.transpose(1, 2, 3, 0).reshape(C, 9, C)).astype(BF)
    # per-sample expert gather; fold the 1/(H*W) mean into W1
    w1t = np.ascontiguousarray(
        (aW1[idx] / float(H * W)).transpose(0, 2, 1))      # [N, C, CH]
    b1g = np.ascontiguousarray(ab1[idx])[:, :, None]       # [N, CH, 1]
    w2t = np.ascontiguousarray(aW2[idx].transpose(0, 2, 1))  # [N, CH, C]
    b2g = np.ascontiguousarray(ab2[idx])[:, :, None]       # [N, C, 1]

    conv_bias = bool(np.any(conv1_b) or np.any(conv2_b))
    nc = _get_program(float(prelu1), float(prelu2), conv_bias)

    xpad = np.zeros((N, C, HP, HP), np.float32)
    xpad[:, :, 1:H + 1, 1:W + 1] = x

    in_maps = []
    for i in range(NCORES):
        sl = slice(i * SPC, (i + 1) * SPC)
        in_maps.append(dict(
            x=xpad[sl], cw1=cw1, cw2=cw2,
            c1b=conv1_b[:, None], c2b=conv2_b[:, None],
            w1t=w1t[sl], b1=b1g[sl], w2t=w2t[sl], b2=b2g[sl],
            ident=np.eye(C, dtype=np.float32)))
    return nc, in_maps


def kernel(**inputs):
    import time
    from concourse.bass_utils import run_bass_kernel_spmd

    nc, in_maps = _prep(**inputs)
    res = None
    for attempt, pause in enumerate((0, 15, 60, 120)):
        if pause:
            time.sleep(pause)
        try:
            res = run_bass_kernel_spmd(nc, in_maps,
                                       core_ids=list(range(NCORES)))
            break
        except Exception:
            # transient NRT_EXEC_UNIT_UNRECOVERABLE (wedged core); retry
            if attempt == 3:
                raise
    return np.concatenate([r["y"] for r in res.results], axis=0)



# revision 13
# speedup vs baseline: 2.3639x; 2.3639x over previous
"""Trainium2 Bass kernel for nn_DomainAdaption (conv-conv-MoE-gated-residual).

Data-parallel over batch: 16 samples -> 8 NeuronCores, 2 samples/core.
Per sample on-device:
  h1 = prelu(conv3x3(x, w1))        fp8e4 DoubleRow matmuls (2 taps/instr,
                                    5 pairs per output row), ScalarE Prelu
                                    drain (scale=1/64) -> fp8 h1 + accum T
  gate: mean(conv3x3(h1, w2)) is, by linearity, an exact function of h1's
        total/row/col marginal sums -> 9 tap-sums S (DVE), x1 = cw2_bf16 @ S
        (PE), adapter MLP (host-gathered expert weights) -> sigmoid gate.
        Gate is thus ready BEFORE conv2 runs.
  out = prelu(g*conv2(h1) + x):     conv2 fp8 DoubleRow pairs accumulate
                                    64*conv2 into PSUM; one fp16 matmul
                                    diag(64/g) @ x16 adds the residual;
                                    ScalarE drain Prelu(psum*(g/64)) -> y f16
No separate residual pass and no tail: the last drain is the end.
"""
import sys

if "/opt/trn_rl_repo" not in sys.path:
    sys.path.insert(0, "/opt/trn_rl_repo")

import numpy as np
import ml_dtypes

N, C, H, W = 16, 128, 128, 128
CH = 32
NCORES = 8
SPC = N // NCORES          # samples per core
HP = H + 2                 # padded rows/cols
NC2 = H // 4               # 32 chunks of 4 rows
SC = 64.0                  # fp8 weight pre-scale
BF = ml_dtypes.bfloat16
E4 = ml_dtypes.float8_e4m3

# tap order for weight pairs (dy,dx); pairs: (0,1),(2,3),(4,5),(6,7),(8,zero)
TAPS = [(0, 0), (1, 0), (0, 1), (1, 1), (0, 2), (1, 2), (2, 0), (2, 1),
        (2, 2)]
# placement of gate-chain PE ops among conv2 pair-chunks
STATS_AT, MM1_AT, MM2_AT = 5, 6, 7
PSC_BUFS = 7               # 1-bank psum tiles; chunks 0..6 open pre-gate


def _build(prelu1: float, prelu2: float, conv_bias: bool):
    import bass_rust
    import concourse.mybir as mybir
    import concourse.tile as tile
    from concourse import bacc

    F32 = mybir.dt.float32
    F16 = mybir.dt.float16
    BF16 = mybir.dt.bfloat16
    F8 = mybir.dt.float8e4
    AF = mybir.ActivationFunctionType
    ALU = mybir.AluOpType
    DR = mybir.MatmulPerfMode.DoubleRow

    nc = bacc.Bacc("TRN2", target_bir_lowering=False, debug=False,
                   num_devices=NCORES)

    x8_d = nc.dram_tensor("x8", [SPC, C, HP, HP], F8, kind="ExternalInput").ap()
    x16_d = nc.dram_tensor("x16", [SPC, C, H, W], F16,
                           kind="ExternalInput").ap()
    cw1_d = nc.dram_tensor("cw1", [C, 10, C], F8, kind="ExternalInput").ap()
    cw2_d = nc.dram_tensor("cw2", [C, 10, C], F8, kind="ExternalInput").ap()
    cwb_d = nc.dram_tensor("cwb", [C, 9, C], BF16, kind="ExternalInput").ap()
    idf_d = nc.dram_tensor("idf", [C, C], F16, kind="ExternalInput").ap()
    # adp1 = [w1T | b2], adp2 = [w2T | b1]  (host-gathered per-sample experts)
    adp1_d = nc.dram_tensor("adp1", [SPC, C, CH + 1], F32,
                            kind="ExternalInput").ap()
    adp2_d = nc.dram_tensor("adp2", [SPC, CH, C + 1], F32,
                            kind="ExternalInput").ap()
    if conv_bias:
        c1b_d = nc.dram_tensor("c1b", [C, 1], F32, kind="ExternalInput").ap()
        c2b_d = nc.dram_tensor("c2b", [C, 1], F32, kind="ExternalInput").ap()
    y_d = nc.dram_tensor("y", [SPC, C, H, W], F16, kind="ExternalOutput").ap()

    def pair2(plane, row, col, pstride):
        """[C, 2, W] fp8 view of `plane` at (row, col): the two DoubleRow
        halves are offset by `pstride` elements (row/col shifted taps)."""
        v = plane[:, row, col:col + W].copy()
        a = [list(p) for p in v.ap]
        v.ap = bass_rust.VecI64Pair([a[0], [pstride, 2], [1, W]])
        return v

    with tile.TileContext(nc) as tc, (
        tc.tile_pool(name="wp", bufs=1)) as wp, (
        tc.tile_pool(name="x8p", bufs=2)) as x8p, (
        tc.tile_pool(name="x16p", bufs=2)) as x16p, (
        tc.tile_pool(name="h1p", bufs=1)) as h1p, (
        tc.tile_pool(name="stp", bufs=2)) as stp, (
        tc.tile_pool(name="adp", bufs=2)) as adp, (
        tc.tile_pool(name="gsm", bufs=1)) as gsm, (
        tc.tile_pool(name="sgp", bufs=2)) as sgp, (
        tc.tile_pool(name="dgp", bufs=2)) as dgp, (
        tc.tile_pool(name="ysp", bufs=3)) as ysp, (
        tc.tile_pool(name="psc", bufs=PSC_BUFS, space="PSUM")) as psc, (
        tc.tile_pool(name="psv", bufs=1, space="PSUM")) as psv:

        cw1_t = wp.tile([C, 10, C], F8)
        cw2_t = wp.tile([C, 10, C], F8)
        cwb_t = wp.tile([C, 9, C], BF16)
        idf_t = wp.tile([C, C], F16)
        if conv_bias:
            c1b_t = wp.tile([C, 1], F32)
            c2b_t = wp.tile([C, 1], F32)

        h1_t = h1p.tile([C, HP, HP], F8)
        nc.vector.memset(h1_t[:, 0, :], 0)
        nc.vector.memset(h1_t[:, HP - 1, :], 0)
        nc.vector.memset(h1_t[:, 1:HP - 1, 0], 0)
        nc.vector.memset(h1_t[:, 1:HP - 1, HP - 1], 0)

        nc.scalar.dma_start(cw1_t[:], cw1_d)
        if conv_bias:
            nc.scalar.dma_start(c1b_t[:], c1b_d)
            nc.scalar.dma_start(c2b_t[:], c2b_d)

        def conv_pairs(pch, plane, weights, k, close):
            """5 DoubleRow pair-matmuls per output row. start=True ONLY on
            the chunk's very first matmul: start marks the whole 2KB PSUM
            zero-region (bank) pending-zero, so a second start inside the
            chunk would wipe earlier rows' accumulated values. close=True
            ends the bank group here; close=False leaves it open for the
            x-residual matmul that accumulates on top later."""
            for j in range(4):
                r = 4 * k + j
                rhs = [plane[:, r:r + 2, 0:W],
                       plane[:, r:r + 2, 1:W + 1],
                       plane[:, r:r + 2, 2:W + 2],
                       pair2(plane, r + 2, 0, 1),
                       pair2(plane, r + 2, 2, -HP)]
                for p in range(5):
                    nc.tensor.matmul(
                        pch[:, j, :], weights[:, 2 * p:2 * p + 2, :], rhs[p],
                        start=(j == 0 and p == 0),
                        stop=(close and j == 3 and p == 4), perf_mode=DR,
                        skip_group_check=True)

        for s in range(SPC):
            xs = x8p.tile([C, HP, HP], F8, name="xs")
            nc.sync.dma_start(xs[:, 0:34, :], x8_d[s, :, 0:34, :])
            nc.sync.dma_start(xs[:, 34:66, :], x8_d[s, :, 34:66, :])
            nc.sync.dma_start(xs[:, 66:98, :], x8_d[s, :, 66:98, :])
            nc.sync.dma_start(xs[:, 98:130, :], x8_d[s, :, 98:130, :])
            x16_t = x16p.tile([C, H, W], F16, name="x16")
            nc.gpsimd.dma_start(x16_t[:, 0:64, :], x16_d[s, :, 0:64, :])
            nc.gpsimd.dma_start(x16_t[:, 64:128, :], x16_d[s, :, 64:128, :])
            adp1_t = adp.tile([C, CH + 1], F32)
            nc.sync.dma_start(adp1_t[:], adp1_d[s])
            adp2_t = adp.tile([CH, C + 1], F32)
            nc.sync.dma_start(adp2_t[:], adp2_d[s])
            if s == 0:
                nc.sync.dma_start(cw2_t[:], cw2_d)
                nc.sync.dma_start(cwb_t[:], cwb_d)
                nc.sync.dma_start(idf_t[:], idf_d)

            # ---- conv1: fp8 pairs -> Prelu drain -> fp8 h1 (+ accum T)
            tacc = stp.tile([C, NC2], F32)
            for k in range(NC2):
                pch = psc.tile([C, 4, W], F32)
                conv_pairs(pch, xs, cw1_t, k, close=True)
                nc.scalar.activation(
                    h1_t[:, 4 * k + 1:4 * k + 5, 1:W + 1], pch[:], AF.Prelu,
                    bias=(c1b_t[:] if conv_bias else 0.0), scale=1.0 / SC,
                    alpha=prelu1, accum_out=tacc[:, k:k + 1])

            # ---- marginal sums of h1 -> 9 tap-sums S (DVE, all [C,1])
            _cid = [0]

            def col(shape=(C, 1), dt=F32):
                _cid[0] += 1
                return gsm.tile(list(shape), dt, name=f"g{s}_{_cid[0]}")

            T = col(); nc.vector.tensor_reduce(
                T[:], tacc[:], axis=mybir.AxisListType.X, op=ALU.add)
            rt = col(); nc.vector.tensor_reduce(
                rt[:], h1_t[:, 1, 1:W + 1], axis=mybir.AxisListType.X,
                op=ALU.add)
            rb = col(); nc.vector.tensor_reduce(
                rb[:], h1_t[:, H, 1:W + 1], axis=mybir.AxisListType.X,
                op=ALU.add)
            cl = col(); nc.vector.tensor_reduce(
                cl[:], h1_t[:, 1:H + 1, 1], axis=mybir.AxisListType.X,
                op=ALU.add)
            cr = col(); nc.vector.tensor_reduce(
                cr[:], h1_t[:, 1:H + 1, W], axis=mybir.AxisListType.X,
                op=ALU.add)
            tl = h1_t[:, 1, 1:2]; tr = h1_t[:, 1, W:W + 1]
            bl = h1_t[:, H, 1:2]; br = h1_t[:, H, W:W + 1]
            A0 = col(); nc.vector.tensor_sub(A0[:], T[:], rb[:])
            A2 = col(); nc.vector.tensor_sub(A2[:], T[:], rt[:])
            S = sgp.tile([C, 9], BF16)
            tmp = col(); nc.vector.tensor_sub(tmp[:], A0[:], cr[:])
            nc.vector.tensor_tensor(S[:, 0:1], tmp[:], br, op=ALU.add)
            nc.vector.tensor_sub(S[:, 1:2], T[:], cr[:])
            nc.vector.tensor_scalar_add(S[:, 2:3], A0[:], 0.0)
            nc.vector.tensor_scalar_add(S[:, 3:4], T[:], 0.0)
            tmp = col(); nc.vector.tensor_sub(tmp[:], A0[:], cl[:])
            nc.vector.tensor_tensor(S[:, 4:5], tmp[:], bl, op=ALU.add)
            nc.vector.tensor_sub(S[:, 5:6], T[:], cl[:])
            tmp = col(); nc.vector.tensor_sub(tmp[:], A2[:], cr[:])
            nc.vector.tensor_tensor(S[:, 6:7], tmp[:], tr, op=ALU.add)
            nc.vector.tensor_scalar_add(S[:, 7:8], A2[:], 0.0)
            tmp = col(); nc.vector.tensor_sub(tmp[:], A2[:], cl[:])
            nc.vector.tensor_tensor(S[:, 8:9], tmp[:], tl, op=ALU.add)

            # ---- conv2 with gate chain interleaved; drains emit final y
            x1c = col(); a_t = col((CH, 1)); gate = col()
            scg = col(); rec = col(); rec64 = col()
            dgt = dgp.tile([C, C], F16)
            if conv_bias:
                bsg = col()
            ystage = None
            pend = []

            def flush_one():
                k, pch = pend.pop(0)
                nonlocal ystage
                if k % 4 == 0:
                    ystage = ysp.tile([C, 16, W], F16)
                nc.tensor.matmul(pch[:, 0:4, :], dgt[:],
                                 x16_t[:, 4 * k:4 * k + 4, :],
                                 start=False, stop=True,
                                 skip_group_check=True)
                nc.scalar.activation(
                    ystage[:, (k % 4) * 4:(k % 4) * 4 + 4, :], pch[:],
                    AF.Prelu, bias=(bsg[:] if conv_bias else 0.0),
                    scale=scg[:], alpha=prelu2)
                if k % 4 == 3:
                    r0 = 16 * (k // 4)
                    dma = nc.sync if (k // 4) % 2 == 0 else nc.gpsimd
                    dma.dma_start(y_d[s, :, r0:r0 + 16, :], ystage[:])

            for k in range(NC2):
                if k == STATS_AT:
                    psx = psv.tile([C, 1], F32, name='psv_t')
                    for t in range(9):
                        nc.tensor.matmul(psx[:], cwb_t[:, t, :], S[:, t:t + 1],
                                         start=(t == 0), stop=(t == 8))
                    nc.scalar.copy(x1c[:], psx[:])
                if k == MM1_AT:
                    psa = psv.tile([C, 1], F32, name='psv_t')
                    nc.tensor.matmul(psa[0:CH, :], adp1_t[:, 0:CH], x1c[:],
                                     start=True, stop=True)
                    nc.vector.tensor_scalar(a_t[:], psa[0:CH, :],
                                            adp2_t[:, C:C + 1], 0.0,
                                            ALU.add, ALU.max)
                if k == MM2_AT:
                    psg = psv.tile([C, 1], F32, name='psv_t')
                    nc.tensor.matmul(psg[:], adp2_t[:, 0:C], a_t[:],
                                     start=True, stop=True)
                    nc.scalar.activation(gate[:], psg[:], AF.Sigmoid,
                                         bias=adp1_t[:, CH:CH + 1])
                    nc.vector.tensor_scalar_mul(scg[:], gate[:], 1.0 / SC)
                    if conv_bias:
                        nc.vector.tensor_scalar_mul(bsg[:], gate[:],
                                                    c2b_t[:])
                    nc.vector.reciprocal(rec[:], gate[:])
                    nc.vector.tensor_scalar_mul(rec64[:], rec[:], SC)
                    nc.vector.tensor_scalar_mul(dgt[:], idf_t[:], rec64[:])
                if len(pend) >= PSC_BUFS:
                    flush_one()
                pch = psc.tile([C, 4, W], F32)
                conv_pairs(pch, h1_t, cw2_t, k, close=False)
                pend.append((k, pch))
            while pend:
                flush_one()

    nc.compile()
    return nc


_CACHE = {}


def _get_program(prelu1, prelu2, conv_bias):
    key = (float(prelu1), float(prelu2), bool(conv_bias))
    if key not in _CACHE:
        _CACHE[key] = _build(*key)
    return _CACHE[key]


def _prep(x, intensity, conv1_w, conv1_b, prelu1, conv2_w, conv2_b,
          aW1, ab1, aW2, ab2, prelu2):
    x = np.asarray(x, np.float32)
    idx = np.asarray(intensity).astype(np.int64) - 1
    conv1_w = np.asarray(conv1_w, np.float32)
    conv1_b = np.asarray(conv1_b, np.float32)
    conv2_w = np.asarray(conv2_w, np.float32)
    conv2_b = np.asarray(conv2_b, np.float32)
    aW1 = np.asarray(aW1, np.float32)
    ab1 = np.asarray(ab1, np.float32)
    aW2 = np.asarray(aW2, np.float32)
    ab2 = np.asarray(ab2, np.float32)

    # [Co,Ci,ky,kx] -> [Ci, tap, Co] in DoubleRow pair order + zero pad tap
    def packtaps(w, scale, dt):
        cw = np.zeros((C, 10, C), dt)
        for t, (dy, dx) in enumerate(TAPS):
            cw[:, t, :] = (w[:, :, dy, dx].T * scale).astype(dt)
        return cw

    cw1 = packtaps(conv1_w, SC, E4)
    cw2 = packtaps(conv2_w, SC, E4)
    cwb = packtaps(conv2_w, 1.0, BF)[:, 0:9, :].copy()

    # per-sample expert gather; fold 1/(H*W) into W1^T and W1@conv2_b into b1
    w1t = np.ascontiguousarray(
        (aW1[idx] / float(H * W)).transpose(0, 2, 1))        # [N, C, CH]
    b1g = ab1[idx] + np.einsum('nhc,c->nh', aW1[idx], conv2_b)  # [N, CH]
    w2t = np.ascontiguousarray(aW2[idx].transpose(0, 2, 1))  # [N, CH, C]
    b2g = ab2[idx]                                           # [N, C]
    adp1 = np.concatenate([w1t, b2g[:, :, None]], axis=2)    # [N, C, CH+1]
    adp2 = np.concatenate([w2t, b1g[:, :, None]], axis=2)    # [N, CH, C+1]

    conv_bias = bool(np.any(conv1_b) or np.any(conv2_b))
    nc = _get_program(float(prelu1), float(prelu2), conv_bias)

    x8 = np.zeros((N, C, HP, HP), E4)
    x8[:, :, 1:H + 1, 1:W + 1] = x.astype(E4)
    x16 = x.astype(np.float16)
    idf = np.eye(C, dtype=np.float16)

    in_maps = []
    for i in range(NCORES):
        sl = slice(i * SPC, (i + 1) * SPC)
        m = dict(x8=x8[sl], x16=x16[sl], cw1=cw1, cw2=cw2, cwb=cwb,
                 idf=idf, adp1=adp1[sl], adp2=adp2[sl])
        if conv_bias:
            m["c1b"] = conv1_b[:, None]
            m["c2b"] = conv2_b[:, None]
        in_maps.append(m)
    return nc, in_maps


def kernel(**inputs):
    import time
    from concourse.bass_utils import run_bass_kernel_spmd

    nc, in_maps = _prep(**inputs)
    res = None
    for attempt, pause in enumerate((0, 15, 60, 120)):
        if pause:
            time.sleep(pause)
        try:
            res = run_bass_kernel_spmd(nc, in_maps,
                                       core_ids=list(range(NCORES)))
            break
        except Exception:
            # transient NRT_EXEC_UNIT_UNRECOVERABLE (wedged core); retry
            if attempt == 3:
                raise
    return np.concatenate(
        [np.asarray(r["y"], np.float32) for r in res.results], axis=0)


# revision 16
# speedup vs baseline: 2.7385x; 1.1585x over previous
"""Trainium2 Bass kernel for nn_DomainAdaption (conv-conv-MoE-gated-residual).

Data-parallel over batch: 16 samples -> 8 NeuronCores, 2 samples/core.
Per sample on-device:
  h1 = prelu(conv3x3(x, w1))        fp8e4 DoubleRow matmuls (2 taps/instr,
                                    5 pairs per output row), ScalarE Prelu
                                    drain (scale=1/64) -> fp8 h1 + accum T
  gate: mean(conv3x3(h1, w2)) is, by linearity, an exact function of h1's
        total/row/col marginal sums -> 9 tap-sums S (DVE), x1 = cw2_bf16 @ S
        (PE), adapter MLP (host-gathered expert weights) -> sigmoid gate.
        Gate is thus ready BEFORE conv2 runs.
  out = prelu(g*conv2(h1) + x):     conv2 fp8 DoubleRow pairs accumulate
                                    64*conv2 into PSUM; one fp16 matmul
                                    diag(64/g) @ x16 adds the residual;
                                    ScalarE drain Prelu(psum*(g/64)) -> y f16
No separate residual pass and no tail: the last drain is the end.
"""
import sys

if "/opt/trn_rl_repo" not in sys.path:
    sys.path.insert(0, "/opt/trn_rl_repo")

import numpy as np
import ml_dtypes

N, C, H, W = 16, 128, 128, 128
CH = 32
NCORES = 8
SPC = N // NCORES          # samples per core
HP = H + 2                 # padded rows/cols
NC2 = H // 4               # 32 chunks of 4 rows
SC = 64.0                  # fp8 weight pre-scale
BF = ml_dtypes.bfloat16
E4 = ml_dtypes.float8_e4m3

# tap order for weight pairs (dy,dx); pairs: (0,1),(2,3),(4,5),(6,7),(8,zero)
TAPS = [(0, 0), (1, 0), (0, 1), (1, 1), (0, 2), (1, 2), (2, 0), (2, 1),
        (2, 2)]
# placement of gate-chain PE ops among conv2 pair-chunks
STATS_AT, MM1_AT, MM2_AT = 5, 6, 7
PSC_BUFS = 7               # 1-bank psum tiles; chunks 0..6 open pre-gate


def _build(prelu1: float, prelu2: float, conv_bias: bool):
    import bass_rust
    import concourse.mybir as mybir
    import concourse.tile as tile
    from concourse import bacc

    F32 = mybir.dt.float32
    F16 = mybir.dt.float16
    BF16 = mybir.dt.bfloat16
    F8 = mybir.dt.float8e4
    AF = mybir.ActivationFunctionType
    ALU = mybir.AluOpType
    DR = mybir.MatmulPerfMode.DoubleRow

    nc = bacc.Bacc("TRN2", target_bir_lowering=False, debug=False,
                   num_devices=NCORES)

    x8_d = nc.dram_tensor("x8", [SPC, C, HP, HP], F8, kind="ExternalInput").ap()
    x16_d = nc.dram_tensor("x16", [SPC, C, H, W], F16,
                           kind="ExternalInput").ap()
    cw1_d = nc.dram_tensor("cw1", [C, 10, C], F8, kind="ExternalInput").ap()
    cw2_d = nc.dram_tensor("cw2", [C, 10, C], F8, kind="ExternalInput").ap()
    cwb_d = nc.dram_tensor("cwb", [C, 9, C], BF16, kind="ExternalInput").ap()
    idf_d = nc.dram_tensor("idf", [C, C], F16, kind="ExternalInput").ap()
    # adp1 = [w1T | b2], adp2 = [w2T | b1]  (host-gathered per-sample experts)
    adp1_d = nc.dram_tensor("adp1", [SPC, C, CH + 1], F32,
                            kind="ExternalInput").ap()
    adp2_d = nc.dram_tensor("adp2", [SPC, CH, C + 1], F32,
                            kind="ExternalInput").ap()
    if conv_bias:
        c1b_d = nc.dram_tensor("c1b", [C, 1], F32, kind="ExternalInput").ap()
        c2b_d = nc.dram_tensor("c2b", [C, 1], F32, kind="ExternalInput").ap()
    y_d = nc.dram_tensor("y", [SPC, C, H, W], F16, kind="ExternalOutput").ap()

    def pair2(plane, row, col, pstride):
        """[C, 2, W] fp8 view of `plane` at (row, col): the two DoubleRow
        halves are offset by `pstride` elements (row/col shifted taps)."""
        v = plane[:, row, col:col + W].copy()
        a = [list(p) for p in v.ap]
        v.ap = bass_rust.VecI64Pair([a[0], [pstride, 2], [1, W]])
        return v

    with tile.TileContext(nc) as tc, (
        tc.tile_pool(name="wp", bufs=1)) as wp, (
        tc.tile_pool(name="x8p", bufs=2)) as x8p, (
        tc.tile_pool(name="x16p", bufs=2)) as x16p, (
        tc.tile_pool(name="h1p", bufs=1)) as h1p, (
        tc.tile_pool(name="stp", bufs=2)) as stp, (
        tc.tile_pool(name="adp", bufs=2)) as adp, (
        tc.tile_pool(name="gsm", bufs=1)) as gsm, (
        tc.tile_pool(name="sgp", bufs=2)) as sgp, (
        tc.tile_pool(name="dgp", bufs=2)) as dgp, (
        tc.tile_pool(name="ysp", bufs=3)) as ysp, (
        tc.tile_pool(name="dvp", bufs=2)) as dvp, (
        tc.tile_pool(name="psc", bufs=PSC_BUFS, space="PSUM")) as psc, (
        tc.tile_pool(name="psv", bufs=1, space="PSUM")) as psv:

        cw1_t = wp.tile([C, 10, C], F8)
        cw2_t = wp.tile([C, 10, C], F8)
        cwb_t = wp.tile([C, 9, C], BF16)
        idf_t = wp.tile([C, C], F16)
        if conv_bias:
            c1b_t = wp.tile([C, 1], F32)
            c2b_t = wp.tile([C, 1], F32)

        h1_t = h1p.tile([C, HP, HP], F8)
        nc.vector.memset(h1_t[:, 0, :], 0)
        nc.vector.memset(h1_t[:, HP - 1, :], 0)
        nc.vector.memset(h1_t[:, 1:HP - 1, 0], 0)
        nc.vector.memset(h1_t[:, 1:HP - 1, HP - 1], 0)

        nc.scalar.dma_start(cw1_t[:], cw1_d)
        if conv_bias:
            nc.scalar.dma_start(c1b_t[:], c1b_d)
            nc.scalar.dma_start(c2b_t[:], c2b_d)

        def conv_pairs(pch, plane, weights, k, close):
            """5 DoubleRow pair-matmuls per output row. start=True ONLY on
            the chunk's very first matmul: start marks the whole 2KB PSUM
            zero-region (bank) pending-zero, so a second start inside the
            chunk would wipe earlier rows' accumulated values. close=True
            ends the bank group here; close=False leaves it open for the
            x-residual matmul that accumulates on top later."""
            for j in range(4):
                r = 4 * k + j
                rhs = [plane[:, r:r + 2, 0:W],
                       plane[:, r:r + 2, 1:W + 1],
                       plane[:, r:r + 2, 2:W + 2],
                       pair2(plane, r + 2, 0, 1),
                       pair2(plane, r + 2, 2, -HP)]
                for p in range(5):
                    nc.tensor.matmul(
                        pch[:, j, :], weights[:, 2 * p:2 * p + 2, :], rhs[p],
                        start=(j == 0 and p == 0),
                        stop=(close and j == 3 and p == 4), perf_mode=DR,
                        skip_group_check=True)

        for s in range(SPC):
            xs = x8p.tile([C, HP, HP], F8, name="xs")
            nc.sync.dma_start(xs[:, 0:12, :], x8_d[s, :, 0:12, :])
            nc.sync.dma_start(xs[:, 12:34, :], x8_d[s, :, 12:34, :])
            nc.sync.dma_start(xs[:, 34:66, :], x8_d[s, :, 34:66, :])
            nc.sync.dma_start(xs[:, 66:98, :], x8_d[s, :, 66:98, :])
            nc.sync.dma_start(xs[:, 98:130, :], x8_d[s, :, 98:130, :])
            x16_t = x16p.tile([C, H, W], F16, name="x16")
            adp1_t = adp.tile([C, CH + 1], F32)
            nc.sync.dma_start(adp1_t[:], adp1_d[s])
            adp2_t = adp.tile([CH, C + 1], F32)
            nc.sync.dma_start(adp2_t[:], adp2_d[s])

            # ---- conv1: fp8 pairs -> Prelu drain -> fp8 h1 (+ accum T).
            # Drains split ScalarE/DVE (DVE: scale+bias TS, then prelu STT)
            # so neither engine paces the phase. x16/weight DMAs are issued
            # mid-loop so they never head-of-line-block the x8 bands on the
            # exclusive DMA engines.
            tacc = stp.tile([C, NC2], F32)
            tmpd = None
            for k in range(NC2):
                if s == 0 and k in (1, 3, 5):
                    dmas = {1: (cw2_t, cw2_d), 3: (cwb_t, cwb_d),
                            5: (idf_t, idf_d)}
                    t_, d_ = dmas[k]
                    nc.sync.dma_start(t_[:], d_)
                if k in (2, 8, 14, 20):
                    q = {2: 0, 8: 1, 14: 2, 20: 3}[k] * 32
                    nc.gpsimd.dma_start(x16_t[:, q:q + 32, :],
                                        x16_d[s, :, q:q + 32, :])
                pch = psc.tile([C, 4, W], F32)
                conv_pairs(pch, xs, cw1_t, k, close=True)
                if k % 8 in (2, 5, 7) and 0.0 <= prelu1 <= 1.0:
                    tmpd = dvp.tile([C, 4, W], F32, name="tmpd")
                    if conv_bias:
                        nc.vector.tensor_scalar(tmpd[:], pch[:], 1.0 / SC,
                                                c1b_t[:], ALU.mult, ALU.add)
                    else:
                        nc.vector.tensor_scalar_mul(tmpd[:], pch[:], 1.0 / SC)
                    # prelu(t) = max(a*t, t) for 0<=a<=1
                    nc.vector.scalar_tensor_tensor(
                        h1_t[:, 4 * k + 1:4 * k + 5, 1:W + 1], tmpd[:],
                        prelu1, tmpd[:], op0=ALU.mult, op1=ALU.max,
                        accum_out=tacc[:, k:k + 1])
                else:
                    nc.scalar.activation(
                        h1_t[:, 4 * k + 1:4 * k + 5, 1:W + 1], pch[:],
                        AF.Prelu, bias=(c1b_t[:] if conv_bias else 0.0),
                        scale=1.0 / SC, alpha=prelu1,
                        accum_out=tacc[:, k:k + 1])

            # ---- marginal sums of h1 -> 9 tap-sums S (DVE, all [C,1])
            _cid = [0]

            def col(shape=(C, 1), dt=F32):
                _cid[0] += 1
                return gsm.tile(list(shape), dt, name=f"g{s}_{_cid[0]}")

            T = col(); nc.vector.tensor_reduce(
                T[:], tacc[:], axis=mybir.AxisListType.X, op=ALU.add)
            rt = col(); nc.vector.tensor_reduce(
                rt[:], h1_t[:, 1, 1:W + 1], axis=mybir.AxisListType.X,
                op=ALU.add)
            rb = col(); nc.vector.tensor_reduce(
                rb[:], h1_t[:, H, 1:W + 1], axis=mybir.AxisListType.X,
                op=ALU.add)
            cl = col(); nc.vector.tensor_reduce(
                cl[:], h1_t[:, 1:H + 1, 1], axis=mybir.AxisListType.X,
                op=ALU.add)
            cr = col(); nc.vector.tensor_reduce(
                cr[:], h1_t[:, 1:H + 1, W], axis=mybir.AxisListType.X,
                op=ALU.add)
            tl = h1_t[:, 1, 1:2]; tr = h1_t[:, 1, W:W + 1]
            bl = h1_t[:, H, 1:2]; br = h1_t[:, H, W:W + 1]
            A0 = col(); nc.vector.tensor_sub(A0[:], T[:], rb[:])
            A2 = col(); nc.vector.tensor_sub(A2[:], T[:], rt[:])
            S = sgp.tile([C, 9], BF16)
            tmp = col(); nc.vector.tensor_sub(tmp[:], A0[:], cr[:])
            nc.vector.tensor_tensor(S[:, 0:1], tmp[:], br, op=ALU.add)
            nc.vector.tensor_sub(S[:, 1:2], T[:], cr[:])
            nc.vector.tensor_scalar_add(S[:, 2:3], A0[:], 0.0)
            nc.vector.tensor_scalar_add(S[:, 3:4], T[:], 0.0)
            tmp = col(); nc.vector.tensor_sub(tmp[:], A0[:], cl[:])
            nc.vector.tensor_tensor(S[:, 4:5], tmp[:], bl, op=ALU.add)
            nc.vector.tensor_sub(S[:, 5:6], T[:], cl[:])
            tmp = col(); nc.vector.tensor_sub(tmp[:], A2[:], cr[:])
            nc.vector.tensor_tensor(S[:, 6:7], tmp[:], tr, op=ALU.add)
            nc.vector.tensor_scalar_add(S[:, 7:8], A2[:], 0.0)
            tmp = col(); nc.vector.tensor_sub(tmp[:], A2[:], cl[:])
            nc.vector.tensor_tensor(S[:, 8:9], tmp[:], tl, op=ALU.add)

            # ---- conv2 with gate chain interleaved; drains emit final y
            x1c = col(); a_t = col((CH, 1)); gate = col()
            scg = col(); rec = col(); rec64 = col()
            dgt = dgp.tile([C, C], F16)
            if conv_bias:
                bsg = col()
            ystage = None
            pend = []

            def flush_one():
                k, pch = pend.pop(0)
                nonlocal ystage
                if k % 4 == 0:
                    ystage = ysp.tile([C, 16, W], F16)
                nc.tensor.matmul(pch[:, 0:4, :], dgt[:],
                                 x16_t[:, 4 * k:4 * k + 4, :],
                                 start=False, stop=True,
                                 skip_group_check=True)
                nc.scalar.activation(
                    ystage[:, (k % 4) * 4:(k % 4) * 4 + 4, :], pch[:],
                    AF.Prelu, bias=(bsg[:] if conv_bias else 0.0),
                    scale=scg[:], alpha=prelu2)
                if k % 4 == 3:
                    r0 = 16 * (k // 4)
                    dma = nc.sync if (k // 4) % 2 == 0 else nc.gpsimd
                    dma.dma_start(y_d[s, :, r0:r0 + 16, :], ystage[:])

            for k in range(NC2):
                if k == STATS_AT:
                    psx = psv.tile([C, 1], F32, name='psv_t')
                    for t in range(9):
                        nc.tensor.matmul(psx[:], cwb_t[:, t, :], S[:, t:t + 1],
                                         start=(t == 0), stop=(t == 8))
                    nc.scalar.copy(x1c[:], psx[:])
                if k == MM1_AT:
                    psa = psv.tile([C, 1], F32, name='psv_t')
                    nc.tensor.matmul(psa[0:CH, :], adp1_t[:, 0:CH], x1c[:],
                                     start=True, stop=True)
                    nc.vector.tensor_scalar(a_t[:], psa[0:CH, :],
                                            adp2_t[:, C:C + 1], 0.0,
                                            ALU.add, ALU.max)
                if k == MM2_AT:
                    psg = psv.tile([C, 1], F32, name='psv_t')
                    nc.tensor.matmul(psg[:], adp2_t[:, 0:C], a_t[:],
                                     start=True, stop=True)
                    nc.scalar.activation(gate[:], psg[:], AF.Sigmoid,
                                         bias=adp1_t[:, CH:CH + 1])
                    nc.vector.tensor_scalar_mul(scg[:], gate[:], 1.0 / SC)
                    if conv_bias:
                        nc.vector.tensor_scalar_mul(bsg[:], gate[:],
                                                    c2b_t[:])
                    nc.vector.reciprocal(rec[:], gate[:])
                    nc.vector.tensor_scalar_mul(rec64[:], rec[:], SC)
                    nc.vector.tensor_scalar_mul(dgt[:], idf_t[:], rec64[:])
                if len(pend) >= PSC_BUFS:
                    flush_one()
                pch = psc.tile([C, 4, W], F32)
                conv_pairs(pch, h1_t, cw2_t, k, close=False)
                pend.append((k, pch))
            while pend:
                flush_one()

    nc.compile()
    return nc


_CACHE = {}


def _get_program(prelu1, prelu2, conv_bias):
    key = (float(prelu1), float(prelu2), bool(conv_bias))
    if key not in _CACHE:
        _CACHE[key] = _build(*key)
    return _CACHE[key]


def _prep(x, intensity, conv1_w, conv1_b, prelu1, conv2_w, conv2_b,
          aW1, ab1, aW2, ab2, prelu2):
    x = np.asarray(x, np.float32)
    idx = np.asarray(intensity).astype(np.int64) - 1
    conv1_w = np.asarray(conv1_w, np.float32)
    conv1_b = np.asarray(conv1_b, np.float32)
    conv2_w = np.asarray(conv2_w, np.float32)
    conv2_b = np.asarray(conv2_b, np.float32)
    aW1 = np.asarray(aW1, np.float32)
    ab1 = np.asarray(ab1, np.float32)
    aW2 = np.asarray(aW2, np.float32)
    ab2 = np.asarray(ab2, np.float32)

    # [Co,Ci,ky,kx] -> [Ci, tap, Co] in DoubleRow pair order + zero pad tap
    def packtaps(w, scale, dt):
        cw = np.zeros((C, 10, C), dt)
        for t, (dy, dx) in enumerate(TAPS):
            cw[:, t, :] = (w[:, :, dy, dx].T * scale).astype(dt)
        return cw

    cw1 = packtaps(conv1_w, SC, E4)
    cw2 = packtaps(conv2_w, SC, E4)
    cwb = packtaps(conv2_w, 1.0, BF)[:, 0:9, :].copy()

    # per-sample expert gather; fold 1/(H*W) into W1^T and W1@conv2_b into b1
    w1t = np.ascontiguousarray(
        (aW1[idx] / float(H * W)).transpose(0, 2, 1))        # [N, C, CH]
    b1g = ab1[idx] + np.einsum('nhc,c->nh', aW1[idx], conv2_b)  # [N, CH]
    w2t = np.ascontiguousarray(aW2[idx].transpose(0, 2, 1))  # [N, CH, C]
    b2g = ab2[idx]                                           # [N, C]
    adp1 = np.concatenate([w1t, b2g[:, :, None]], axis=2)    # [N, C, CH+1]
    adp2 = np.concatenate([w2t, b1g[:, :, None]], axis=2)    # [N, CH, C+1]

    conv_bias = bool(np.any(conv1_b) or np.any(conv2_b))
    nc = _get_program(float(prelu1), float(prelu2), conv_bias)

    x8 = np.zeros((N, C, HP, HP), E4)
    x8[:, :, 1:H + 1, 1:W + 1] = x.astype(E4)
    x16 = x.astype(np.float16)
    idf = np.eye(C, dtype=np.float16)

    in_maps = []
    for i in range(NCORES):
        sl = slice(i * SPC, (i + 1) * SPC)
        m = dict(x8=x8[sl], x16=x16[sl], cw1=cw1, cw2=cw2, cwb=cwb,
                 idf=idf, adp1=adp1[sl], adp2=adp2[sl])
        if conv_bias:
            m["c1b"] = conv1_b[:, None]
            m["c2b"] = conv2_b[:, None]
        in_maps.append(m)
    return nc, in_maps


def kernel(**inputs):
    import time
    from concourse.bass_utils import run_bass_kernel_spmd

    nc, in_maps = _prep(**inputs)
    res = None
    for attempt, pause in enumerate((0, 15, 60, 120)):
        if pause:
            time.sleep(pause)
        try:
            res = run_bass_kernel_spmd(nc, in_maps,
                                       core_ids=list(range(NCORES)))
            break
        except Exception:
            # transient NRT_EXEC_UNIT_UNRECOVERABLE (wedged core); retry
            if attempt == 3:
                raise
    return np.concatenate(
        [np.asarray(r["y"], np.float32) for r in res.results], axis=0)


# revision 22
# speedup vs baseline: 2.8380x; 1.0363x over previous
"""Trainium2 Bass kernel for nn_DomainAdaption (conv-conv-MoE-gated-residual).

Data-parallel over batch: 16 samples -> 8 NeuronCores, 2 samples/core.
Per sample on-device:
  h1 = prelu(conv3x3(x, w1))        fp8e4 DoubleRow matmuls (2 taps/instr,
                                    5 pairs per output row), ScalarE Prelu
                                    drain (scale=1/64) -> fp8 h1 + accum T
  gate: mean(conv3x3(h1, w2)) is, by linearity, an exact function of h1's
        total/row/col marginal sums -> 9 tap-sums S (DVE), x1 = cw2_bf16 @ S
        (PE), adapter MLP (host-gathered expert weights) -> sigmoid gate.
        Gate is thus ready BEFORE conv2 runs.
  out = prelu(g*conv2(h1) + x):     conv2 fp8 DoubleRow pairs accumulate
                                    64*conv2 into PSUM; one fp16 matmul
                                    diag(64/g) @ x16 adds the residual;
                                    ScalarE drain Prelu(psum*(g/64)) -> y f16
No separate residual pass and no tail: the last drain is the end.
"""
import sys

if "/opt/trn_rl_repo" not in sys.path:
    sys.path.insert(0, "/opt/trn_rl_repo")

import numpy as np
import ml_dtypes

N, C, H, W = 16, 128, 128, 128
CH = 32
NCORES = 8
SPC = N // NCORES          # samples per core
HP = H + 2                 # padded rows/cols
NC2 = H // 4               # 32 chunks of 4 rows
SC = 64.0                  # fp8 weight pre-scale
BF = ml_dtypes.bfloat16
E4 = ml_dtypes.float8_e4m3

# tap order for weight pairs (dy,dx); pairs: (0,1),(2,3),(4,5),(6,7),(8,9)
# slot 8 is the ZERO pad tap (first half of the last pair) so every pair's
# rhs view uses positive strides only (keeps subtile dep tracking exact)
TAPS = [(0, 0), (1, 0), (0, 1), (1, 1), (0, 2), (1, 2), (2, 0), (2, 1),
        None, (2, 2)]
# placement of gate-chain PE ops among conv2 pair-chunks
STATS_AT, MM1_AT, MM2_AT = 6, 7, 8
PRE = 4                    # conv2 chunks drained ungated (DVE applies gate)
PSC_BUFS = 7               # 1-bank psum tiles


def _build(prelu1: float, prelu2: float, conv_bias: bool):
    import bass_rust
    import concourse.mybir as mybir
    import concourse.tile as tile
    from concourse import bacc

    F32 = mybir.dt.float32
    F16 = mybir.dt.float16
    BF16 = mybir.dt.bfloat16
    F8 = mybir.dt.float8e4
    AF = mybir.ActivationFunctionType
    ALU = mybir.AluOpType
    DR = mybir.MatmulPerfMode.DoubleRow

    nc = bacc.Bacc("TRN2", target_bir_lowering=False, debug=False,
                   num_devices=NCORES)

    x8_d = nc.dram_tensor("x8", [SPC, C, HP, HP], F8, kind="ExternalInput").ap()
    x16_d = nc.dram_tensor("x16", [SPC, C, H, W], F16,
                           kind="ExternalInput").ap()
    cw1_d = nc.dram_tensor("cw1", [C, 10, C], F8, kind="ExternalInput").ap()
    cw2_d = nc.dram_tensor("cw2", [C, 10, C], F8, kind="ExternalInput").ap()
    cwb_d = nc.dram_tensor("cwb", [C, 9, C], BF16, kind="ExternalInput").ap()
    idf_d = nc.dram_tensor("idf", [C, C], F16, kind="ExternalInput").ap()
    # adp1 = [w1T | b2], adp2 = [w2T | b1]  (host-gathered per-sample experts)
    adp1_d = nc.dram_tensor("adp1", [SPC, C, CH + 1], F32,
                            kind="ExternalInput").ap()
    adp2_d = nc.dram_tensor("adp2", [SPC, CH, C + 1], F32,
                            kind="ExternalInput").ap()
    if conv_bias:
        c1b_d = nc.dram_tensor("c1b", [C, 1], F32, kind="ExternalInput").ap()
        c2b_d = nc.dram_tensor("c2b", [C, 1], F32, kind="ExternalInput").ap()
    y_d = nc.dram_tensor("y", [SPC, C, H, W], F16, kind="ExternalOutput").ap()

    def pair2(plane, row, col, pstride):
        """[C, 2, W] fp8 view of `plane` at (row, col): the two DoubleRow
        halves are offset by `pstride` elements (row/col shifted taps)."""
        v = plane[:, row, col:col + W].copy()
        a = [list(p) for p in v.ap]
        v.ap = bass_rust.VecI64Pair([a[0], [pstride, 2], [1, W]])
        return v

    with tile.TileContext(nc) as tc, (
        tc.tile_pool(name="wp", bufs=1)) as wp, (
        tc.tile_pool(name="x8p", bufs=2)) as x8p, (
        tc.tile_pool(name="x16p", bufs=2)) as x16p, (
        tc.tile_pool(name="h1p", bufs=1)) as h1p, (
        tc.tile_pool(name="stp", bufs=2)) as stp, (
        tc.tile_pool(name="adp", bufs=2)) as adp, (
        tc.tile_pool(name="gsm", bufs=1)) as gsm, (
        tc.tile_pool(name="sgp", bufs=2)) as sgp, (
        tc.tile_pool(name="dgp", bufs=2)) as dgp, (
        tc.tile_pool(name="ysp", bufs=3)) as ysp, (
        tc.tile_pool(name="dvp", bufs=2)) as dvp, (
        tc.tile_pool(name="psc", bufs=PSC_BUFS, space="PSUM")) as psc, (
        tc.tile_pool(name="psv", bufs=1, space="PSUM")) as psv:

        cw1_t = wp.tile([C, 10, C], F8)
        cw2_t = wp.tile([C, 10, C], F8)
        cwb_t = wp.tile([C, 9, C], BF16)
        idf_t = wp.tile([C, C], F16)
        if conv_bias:
            c1b_t = wp.tile([C, 1], F32)
            c2b_t = wp.tile([C, 1], F32)

        h1_t = h1p.tile([C, HP, HP], F8)
        nc.vector.memset(h1_t[:, 0, :], 0)
        nc.vector.memset(h1_t[:, HP - 1, :], 0)
        nc.vector.memset(h1_t[:, 1:HP - 1, 0], 0)
        nc.vector.memset(h1_t[:, 1:HP - 1, HP - 1], 0)

        nc.scalar.dma_start(cw1_t[:], cw1_d)
        if conv_bias:
            nc.scalar.dma_start(c1b_t[:], c1b_d)
            nc.scalar.dma_start(c2b_t[:], c2b_d)

        def conv_pairs(pch, plane, weights, k, close):
            """5 DoubleRow pair-matmuls per output row. start=True ONLY on
            the chunk's very first matmul: start marks the whole 2KB PSUM
            zero-region (bank) pending-zero, so a second start inside the
            chunk would wipe earlier rows' accumulated values. close=True
            ends the bank group here; close=False leaves it open for the
            x-residual matmul that accumulates on top later."""
            for j in range(4):
                r = 4 * k + j
                rhs = [plane[:, r:r + 2, 0:W],
                       plane[:, r:r + 2, 1:W + 1],
                       plane[:, r:r + 2, 2:W + 2],
                       pair2(plane, r + 2, 0, 1),
                       pair2(plane, r + 1, 2, HP)]
                for p in range(5):
                    nc.tensor.matmul(
                        pch[:, j, :], weights[:, 2 * p:2 * p + 2, :], rhs[p],
                        start=(j == 0 and p == 0),
                        stop=(close and j == 3 and p == 4), perf_mode=DR,
                        skip_group_check=True)

        for s in range(SPC):
            xs = x8p.tile([C, HP, HP], F8, name="xs")
            nc.sync.dma_start(xs[:, 0:12, :], x8_d[s, :, 0:12, :])
            nc.sync.dma_start(xs[:, 12:34, :], x8_d[s, :, 12:34, :])
            nc.sync.dma_start(xs[:, 34:66, :], x8_d[s, :, 34:66, :])
            nc.sync.dma_start(xs[:, 66:98, :], x8_d[s, :, 66:98, :])
            nc.sync.dma_start(xs[:, 98:130, :], x8_d[s, :, 98:130, :])
            x16_t = x16p.tile([C, H, W], F16, name="x16")
            adp1_t = adp.tile([C, CH + 1], F32)
            nc.sync.dma_start(adp1_t[:], adp1_d[s])
            adp2_t = adp.tile([CH, C + 1], F32)
            nc.sync.dma_start(adp2_t[:], adp2_d[s])

            # ---- conv1: fp8 pairs -> Prelu drain -> fp8 h1 (+ accum T).
            # Drains split ScalarE/DVE (DVE: scale+bias TS, then prelu STT)
            # so neither engine paces the phase. x16/weight DMAs are issued
            # mid-loop so they never head-of-line-block the x8 bands on the
            # exclusive DMA engines.
            tacc = stp.tile([C, NC2], F32)
            tmpd = None
            for k in range(NC2):
                if s == 0 and k in (1, 3, 5):
                    dmas = {1: (cw2_t, cw2_d), 3: (cwb_t, cwb_d),
                            5: (idf_t, idf_d)}
                    t_, d_ = dmas[k]
                    nc.sync.dma_start(t_[:], d_)
                if k in (2, 8, 14, 20):
                    q = {2: 0, 8: 1, 14: 2, 20: 3}[k] * 32
                    nc.gpsimd.dma_start(x16_t[:, q:q + 32, :],
                                        x16_d[s, :, q:q + 32, :])
                pch = psc.tile([C, 4, W], F32)
                conv_pairs(pch, xs, cw1_t, k, close=True)
                if k % 8 in (2, 5, 7) and 0.0 <= prelu1 <= 1.0:
                    tmpd = dvp.tile([C, 4, W], F32, name="tmpd")
                    if conv_bias:
                        nc.vector.tensor_scalar(tmpd[:], pch[:], 1.0 / SC,
                                                c1b_t[:], ALU.mult, ALU.add)
                    else:
                        nc.vector.tensor_scalar_mul(tmpd[:], pch[:], 1.0 / SC)
                    # prelu(t) = max(a*t, t) for 0<=a<=1
                    nc.vector.scalar_tensor_tensor(
                        h1_t[:, 4 * k + 1:4 * k + 5, 1:W + 1], tmpd[:],
                        prelu1, tmpd[:], op0=ALU.mult, op1=ALU.max,
                        accum_out=tacc[:, k:k + 1])
                else:
                    nc.scalar.activation(
                        h1_t[:, 4 * k + 1:4 * k + 5, 1:W + 1], pch[:],
                        AF.Prelu, bias=(c1b_t[:] if conv_bias else 0.0),
                        scale=1.0 / SC, alpha=prelu1,
                        accum_out=tacc[:, k:k + 1])

            # ---- marginal sums of h1 -> 9 tap-sums S (DVE, all [C,1])
            _cid = [0]

            def col(shape=(C, 1), dt=F32):
                _cid[0] += 1
                return gsm.tile(list(shape), dt, name=f"g{s}_{_cid[0]}")

            T = col(); nc.vector.tensor_reduce(
                T[:], tacc[:], axis=mybir.AxisListType.X, op=ALU.add)
            rt = col(); nc.vector.tensor_reduce(
                rt[:], h1_t[:, 1, 1:W + 1], axis=mybir.AxisListType.X,
                op=ALU.add)
            rb = col(); nc.vector.tensor_reduce(
                rb[:], h1_t[:, H, 1:W + 1], axis=mybir.AxisListType.X,
                op=ALU.add)
            cl = col(); nc.vector.tensor_reduce(
                cl[:], h1_t[:, 1:H + 1, 1], axis=mybir.AxisListType.X,
                op=ALU.add)
            cr = col(); nc.vector.tensor_reduce(
                cr[:], h1_t[:, 1:H + 1, W], axis=mybir.AxisListType.X,
                op=ALU.add)
            tl = h1_t[:, 1, 1:2]; tr = h1_t[:, 1, W:W + 1]
            bl = h1_t[:, H, 1:2]; br = h1_t[:, H, W:W + 1]
            A0 = col(); nc.vector.tensor_sub(A0[:], T[:], rb[:])
            A2 = col(); nc.vector.tensor_sub(A2[:], T[:], rt[:])
            S = sgp.tile([C, 9], BF16)
            tmp = col(); nc.vector.tensor_sub(tmp[:], A0[:], cr[:])
            nc.vector.tensor_tensor(S[:, 0:1], tmp[:], br, op=ALU.add)
            nc.vector.tensor_sub(S[:, 1:2], T[:], cr[:])
            nc.vector.tensor_scalar_add(S[:, 2:3], A0[:], 0.0)
            nc.vector.tensor_scalar_add(S[:, 3:4], T[:], 0.0)
            tmp = col(); nc.vector.tensor_sub(tmp[:], A0[:], cl[:])
            nc.vector.tensor_tensor(S[:, 4:5], tmp[:], bl, op=ALU.add)
            nc.vector.tensor_sub(S[:, 5:6], T[:], cl[:])
            tmp = col(); nc.vector.tensor_sub(tmp[:], A2[:], cr[:])
            nc.vector.tensor_tensor(S[:, 6:7], tmp[:], tr, op=ALU.add)
            nc.vector.tensor_scalar_add(S[:, 7:8], A2[:], 0.0)
            tmp = col(); nc.vector.tensor_sub(tmp[:], A2[:], cl[:])
            nc.vector.tensor_tensor(S[:, 8:9], tmp[:], tl, op=ALU.add)

            # ---- conv2 with gate chain interleaved; drains emit final y.
            # The first PRE chunks close ungated (h2 -> bf16 tmp, Act) and
            # DVE applies gate+residual+prelu later: no psum sits open
            # waiting for the gate, so PE never stalls on the gate chain.
            pre = PRE if 0.0 <= prelu2 <= 1.0 else 0
            x1c = col(); a_t = col((CH, 1)); gate = col()
            scg = col(); rec = col(); rec64 = col()
            dgt = dgp.tile([C, C], F16)
            h2t = dgp.tile([C, 4 * PRE, W], BF16)
            if conv_bias:
                bsg = col()
            ystage = None
            pend = []

            def flush_one():
                k, pch = pend.pop(0)
                nonlocal ystage
                if k % 4 == 0:
                    ystage = ysp.tile([C, 16, W], F16)
                nc.tensor.matmul(pch[:, 0:4, :], dgt[:],
                                 x16_t[:, 4 * k:4 * k + 4, :],
                                 start=False, stop=True,
                                 skip_group_check=True)
                nc.scalar.activation(
                    ystage[:, (k % 4) * 4:(k % 4) * 4 + 4, :], pch[:],
                    AF.Prelu, bias=(bsg[:] if conv_bias else 0.0),
                    scale=scg[:], alpha=prelu2)
                if k % 4 == 3:
                    r0 = 16 * (k // 4)
                    dma = nc.sync if (k // 4) % 2 == 0 else nc.gpsimd
                    dma.dma_start(y_d[s, :, r0:r0 + 16, :], ystage[:])

            for k in range(NC2):
                if k == STATS_AT:
                    psx = psv.tile([C, 1], F32, name='psv_t')
                    for t in range(9):
                        nc.tensor.matmul(psx[:], cwb_t[:, t, :], S[:, t:t + 1],
                                         start=(t == 0), stop=(t == 8))
                    nc.scalar.copy(x1c[:], psx[:])
                if k == MM1_AT:
                    psa = psv.tile([C, 1], F32, name='psv_t')
                    nc.tensor.matmul(psa[0:CH, :], adp1_t[:, 0:CH], x1c[:],
                                     start=True, stop=True)
                    nc.vector.tensor_scalar(a_t[:], psa[0:CH, :],
                                            adp2_t[:, C:C + 1], 0.0,
                                            ALU.add, ALU.max)
                if k == MM2_AT:
                    psg = psv.tile([C, 1], F32, name='psv_t')
                    nc.tensor.matmul(psg[:], adp2_t[:, 0:C], a_t[:],
                                     start=True, stop=True)
                    nc.scalar.activation(gate[:], psg[:], AF.Sigmoid,
                                         bias=adp1_t[:, CH:CH + 1])
                    nc.vector.tensor_scalar_mul(scg[:], gate[:], 1.0 / SC)
                    if conv_bias:
                        nc.vector.tensor_scalar_mul(bsg[:], gate[:],
                                                    c2b_t[:])
                    nc.vector.reciprocal(rec[:], gate[:])
                    nc.vector.tensor_scalar_mul(rec64[:], rec[:], SC)
                    nc.vector.tensor_scalar_mul(dgt[:], idf_t[:], rec64[:])
                if k == MM2_AT + 2 and pre:
                    # DVE finishes the PRE ungated chunks: gate, residual,
                    # prelu -> y rows 0..4*PRE (its own ystage group(s))
                    for kp in range(0, pre, 4):
                        yst0 = ysp.tile([C, 16, W], F16, name="ystage")
                        for k4 in range(kp, min(kp + 4, pre)):
                            tt = dvp.tile([C, 4, W], F16, name="tt")
                            nc.vector.scalar_tensor_tensor(
                                tt[:], h2t[:, 4 * k4:4 * k4 + 4, :], gate[:],
                                x16_t[:, 4 * k4:4 * k4 + 4, :],
                                op0=ALU.mult, op1=ALU.add)
                            nc.vector.scalar_tensor_tensor(
                                yst0[:, (k4 % 4) * 4:(k4 % 4) * 4 + 4, :],
                                tt[:], prelu2, tt[:],
                                op0=ALU.mult, op1=ALU.max)
                        nc.gpsimd.dma_start(
                            y_d[s, :, 4 * kp:4 * kp + 16, :], yst0[:])
                while len(pend) >= (PSC_BUFS if k <= MM2_AT + 2 else 2):
                    flush_one()
                pch = psc.tile([C, 4, W], F32)
                if k < pre:
                    conv_pairs(pch, h1_t, cw2_t, k, close=True)
                    nc.scalar.activation(
                        h2t[:, 4 * k:4 * k + 4, :], pch[:], AF.Identity,
                        bias=(c2b_t[:] if conv_bias else 0.0), scale=1.0 / SC)
                else:
                    conv_pairs(pch, h1_t, cw2_t, k, close=False)
                    pend.append((k, pch))
            while pend:
                flush_one()

    nc.compile()
    return nc


_CACHE = {}


def _get_program(prelu1, prelu2, conv_bias):
    key = (float(prelu1), float(prelu2), bool(conv_bias))
    if key not in _CACHE:
        _CACHE[key] = _build(*key)
    return _CACHE[key]


def _prep(x, intensity, conv1_w, conv1_b, prelu1, conv2_w, conv2_b,
          aW1, ab1, aW2, ab2, prelu2):
    x = np.asarray(x, np.float32)
    idx = np.asarray(intensity).astype(np.int64) - 1
    conv1_w = np.asarray(conv1_w, np.float32)
    conv1_b = np.asarray(conv1_b, np.float32)
    conv2_w = np.asarray(conv2_w, np.float32)
    conv2_b = np.asarray(conv2_b, np.float32)
    aW1 = np.asarray(aW1, np.float32)
    ab1 = np.asarray(ab1, np.float32)
    aW2 = np.asarray(aW2, np.float32)
    ab2 = np.asarray(ab2, np.float32)

    # [Co,Ci,ky,kx] -> [Ci, tap, Co] in DoubleRow pair order + zero pad tap
    def packtaps(w, scale, dt, taps):
        cw = np.zeros((C, len(taps), C), dt)
        for t, tap in enumerate(taps):
            if tap is None:
                continue
            dy, dx = tap
            cw[:, t, :] = (w[:, :, dy, dx].T * scale).astype(dt)
        return cw

    cw1 = packtaps(conv1_w, SC, E4, TAPS)
    cw2 = packtaps(conv2_w, SC, E4, TAPS)
    # stats-path taps: the 9 real taps in (row-major over dy of pairs) order
    cwb = packtaps(conv2_w, 1.0, BF, [t for t in TAPS if t is not None])

    # per-sample expert gather; fold 1/(H*W) into W1^T and W1@conv2_b into b1
    w1t = np.ascontiguousarray(
        (aW1[idx] / float(H * W)).transpose(0, 2, 1))        # [N, C, CH]
    b1g = ab1[idx] + np.einsum('nhc,c->nh', aW1[idx], conv2_b)  # [N, CH]
    w2t = np.ascontiguousarray(aW2[idx].transpose(0, 2, 1))  # [N, CH, C]
    b2g = ab2[idx]                                           # [N, C]
    adp1 = np.concatenate([w1t, b2g[:, :, None]], axis=2)    # [N, C, CH+1]
    adp2 = np.concatenate([w2t, b1g[:, :, None]], axis=2)    # [N, CH, C+1]

    conv_bias = bool(np.any(conv1_b) or np.any(conv2_b))
    nc = _get_program(float(prelu1), float(prelu2), conv_bias)

    x8 = np.zeros((N, C, HP, HP), E4)
    x8[:, :, 1:H + 1, 1:W + 1] = x.astype(E4)
    x16 = x.astype(np.float16)
    idf = np.eye(C, dtype=np.float16)

    in_maps = []
    for i in range(NCORES):
        sl = slice(i * SPC, (i + 1) * SPC)
        m = dict(x8=x8[sl], x16=x16[sl], cw1=cw1, cw2=cw2, cwb=cwb,
                 idf=idf, adp1=adp1[sl], adp2=adp2[sl])
        if conv_bias:
            m["c1b"] = conv1_b[:, None]
            m["c2b"] = conv2_b[:, None]
        in_maps.append(m)
    return nc, in_maps


def kernel(**inputs):
    import time
    from concourse.bass_utils import run_bass_kernel_spmd

    nc, in_maps = _prep(**inputs)
    res = None
    for attempt, pause in enumerate((0, 15, 60, 120)):
        if pause:
            time.sleep(pause)
        try:
            res = run_bass_kernel_spmd(nc, in_maps,
                                       core_ids=list(range(NCORES)))
            break
        except Exception:
            # transient NRT_EXEC_UNIT_UNRECOVERABLE (wedged core); retry
            if attempt == 3:
                raise
    return np.concatenate(
        [np.asarray(r["y"], np.float32) for r in res.results], axis=0)


# revision 26
# speedup vs baseline: 2.9972x; 1.0561x over previous
"""Trainium2 Bass kernel for nn_DomainAdaption (conv-conv-MoE-gated-residual).

Data-parallel over batch: 16 samples -> 8 NeuronCores, 2 samples/core.
Per sample on-device:
  h1 = prelu(conv3x3(x, w1))        fp8e4 DoubleRow matmuls (2 taps/instr,
                                    5 pairs per output row), ScalarE Prelu
                                    drain (scale=1/64) -> fp8 h1 + accum T
  gate: mean(conv3x3(h1, w2)) is, by linearity, an exact function of h1's
        total/row/col marginal sums -> 9 tap-sums S (DVE), x1 = cw2_bf16 @ S
        (PE), adapter MLP (host-gathered expert weights) -> sigmoid gate.
        Gate is thus ready BEFORE conv2 runs.
  out = prelu(g*conv2(h1) + x):     conv2 fp8 DoubleRow pairs accumulate
                                    64*conv2 into PSUM; one fp16 matmul
                                    diag(64/g) @ x16 adds the residual;
                                    ScalarE drain Prelu(psum*(g/64)) -> y f16
No separate residual pass and no tail: the last drain is the end.
"""
import sys

if "/opt/trn_rl_repo" not in sys.path:
    sys.path.insert(0, "/opt/trn_rl_repo")

import numpy as np
import ml_dtypes

N, C, H, W = 16, 128, 128, 128
CH = 32
NCORES = 8
SPC = N // NCORES          # samples per core
HP = H + 2                 # padded rows/cols
NC2 = H // 4               # 32 chunks of 4 rows
SC = 64.0                  # fp8 weight pre-scale
BF = ml_dtypes.bfloat16
E4 = ml_dtypes.float8_e4m3

# tap order for weight pairs (dy,dx); pairs: (0,1),(2,3),(4,5),(6,7),(8,9)
# slot 8 is the ZERO pad tap (first half of the last pair) so every pair's
# rhs view uses positive strides only (keeps subtile dep tracking exact)
TAPS = [(0, 0), (1, 0), (0, 1), (1, 1), (0, 2), (1, 2), (2, 0), (2, 1),
        None, (2, 2)]
# placement of gate-chain PE ops among conv2 pair-chunks
STATS_AT, MM1_AT, MM2_AT = 6, 7, 8
PRE = 4                    # conv2 chunks drained ungated (DVE applies gate)
PSC_BUFS = 7               # 1-bank psum tiles


def _build(prelu1: float, prelu2: float, conv_bias: bool):
    import bass_rust
    import concourse.mybir as mybir
    import concourse.tile as tile
    from concourse import bacc

    F32 = mybir.dt.float32
    F16 = mybir.dt.float16
    BF16 = mybir.dt.bfloat16
    F8 = mybir.dt.float8e4
    AF = mybir.ActivationFunctionType
    ALU = mybir.AluOpType
    DR = mybir.MatmulPerfMode.DoubleRow

    nc = bacc.Bacc("TRN2", target_bir_lowering=False, debug=False,
                   num_devices=NCORES)

    x8_d = nc.dram_tensor("x8", [SPC, C, HP, HP], F8, kind="ExternalInput").ap()
    x16_d = nc.dram_tensor("x16", [SPC, C, H, W], F16,
                           kind="ExternalInput").ap()
    cw1_d = nc.dram_tensor("cw1", [C, 10, C], F8, kind="ExternalInput").ap()
    cw2_d = nc.dram_tensor("cw2", [C, 10, C], F8, kind="ExternalInput").ap()
    cwb_d = nc.dram_tensor("cwb", [C, 9, C], BF16, kind="ExternalInput").ap()
    idf_d = nc.dram_tensor("idf", [C, C], F16, kind="ExternalInput").ap()
    # adp1 = [w1T | b2], adp2 = [w2T | b1]  (host-gathered per-sample experts)
    adp1_d = nc.dram_tensor("adp1", [SPC, C, CH + 1], F32,
                            kind="ExternalInput").ap()
    adp2_d = nc.dram_tensor("adp2", [SPC, CH, C + 1], F32,
                            kind="ExternalInput").ap()
    if conv_bias:
        c1b_d = nc.dram_tensor("c1b", [C, 1], F32, kind="ExternalInput").ap()
        c2b_d = nc.dram_tensor("c2b", [C, 1], F32, kind="ExternalInput").ap()
    y_d = nc.dram_tensor("y", [SPC, C, H, W], F16, kind="ExternalOutput").ap()

    def pair2(plane, row, col, pstride):
        """[C, 2, W] fp8 view of `plane` at (row, col): the two DoubleRow
        halves are offset by `pstride` elements (row/col shifted taps)."""
        v = plane[:, row, col:col + W].copy()
        a = [list(p) for p in v.ap]
        v.ap = bass_rust.VecI64Pair([a[0], [pstride, 2], [1, W]])
        return v

    with tile.TileContext(nc) as tc, (
        tc.tile_pool(name="wp", bufs=1)) as wp, (
        tc.tile_pool(name="x8p", bufs=2)) as x8p, (
        tc.tile_pool(name="x16p", bufs=2)) as x16p, (
        tc.tile_pool(name="h1p", bufs=1)) as h1p, (
        tc.tile_pool(name="stp", bufs=2)) as stp, (
        tc.tile_pool(name="adp", bufs=2)) as adp, (
        tc.tile_pool(name="gsm", bufs=1)) as gsm, (
        tc.tile_pool(name="sgp", bufs=2)) as sgp, (
        tc.tile_pool(name="dgp", bufs=2)) as dgp, (
        tc.tile_pool(name="ysp", bufs=3)) as ysp, (
        tc.tile_pool(name="dvp", bufs=2)) as dvp, (
        tc.tile_pool(name="psc", bufs=PSC_BUFS, space="PSUM")) as psc, (
        tc.tile_pool(name="psv", bufs=1, space="PSUM")) as psv:

        cw1_t = wp.tile([C, 10, C], F8)
        cw2_t = wp.tile([C, 10, C], F8)
        cwb_t = wp.tile([C, 9, C], BF16)
        idf_t = wp.tile([C, C], F16)
        if conv_bias:
            c1b_t = wp.tile([C, 1], F32)
            c2b_t = wp.tile([C, 1], F32)

        h1_t = h1p.tile([C, HP, HP], F8)
        nc.vector.memset(h1_t[:, 0, :], 0)
        nc.vector.memset(h1_t[:, HP - 1, :], 0)
        nc.vector.memset(h1_t[:, 1:HP - 1, 0], 0)
        nc.vector.memset(h1_t[:, 1:HP - 1, HP - 1], 0)

        nc.scalar.dma_start(cw1_t[:], cw1_d)
        if conv_bias:
            nc.scalar.dma_start(c1b_t[:], c1b_d)
            nc.scalar.dma_start(c2b_t[:], c2b_d)

        def conv_pairs(pch, plane, weights, k, close):
            """5 DoubleRow pair-matmuls per output row. start=True ONLY on
            the chunk's very first matmul: start marks the whole 2KB PSUM
            zero-region (bank) pending-zero, so a second start inside the
            chunk would wipe earlier rows' accumulated values. close=True
            ends the bank group here; close=False leaves it open for the
            x-residual matmul that accumulates on top later."""
            for j in range(4):
                r = 4 * k + j
                rhs = [plane[:, r:r + 2, 0:W],
                       plane[:, r:r + 2, 1:W + 1],
                       plane[:, r:r + 2, 2:W + 2],
                       pair2(plane, r + 2, 0, 1),
                       pair2(plane, r + 1, 2, HP)]
                for p in range(5):
                    nc.tensor.matmul(
                        pch[:, j, :], weights[:, 2 * p:2 * p + 2, :], rhs[p],
                        start=(j == 0 and p == 0),
                        stop=(close and j == 3 and p == 4), perf_mode=DR,
                        skip_group_check=True)

        xs_next = None
        for s in range(SPC):
            if xs_next is None:
                xs = x8p.tile([C, HP, HP], F8, name="xs")
                nc.sync.dma_start(xs[:, 0:16, :], x8_d[s, :, 0:16, :])
                nc.sync.dma_start(xs[:, 16:34, :], x8_d[s, :, 16:34, :])
                nc.sync.dma_start(xs[:, 34:66, :], x8_d[s, :, 34:66, :])
                nc.sync.dma_start(xs[:, 66:98, :], x8_d[s, :, 66:98, :])
                nc.sync.dma_start(xs[:, 98:130, :], x8_d[s, :, 98:130, :])
            else:
                xs = xs_next
            x16_t = x16p.tile([C, H, W], F16, name="x16")
            adp1_t = adp.tile([C, CH + 1], F32)
            nc.sync.dma_start(adp1_t[:], adp1_d[s])
            adp2_t = adp.tile([CH, C + 1], F32)
            nc.sync.dma_start(adp2_t[:], adp2_d[s])

            # ---- conv1: fp8 pairs -> Prelu drain -> fp8 h1 (+ accum T).
            # Drains split ScalarE/DVE (DVE: scale+bias TS, then prelu STT)
            # so neither engine paces the phase. x16/weight DMAs are issued
            # mid-loop so they never head-of-line-block the x8 bands on the
            # exclusive DMA engines.
            tacc = stp.tile([C, NC2], F32)
            tmpd = None
            for k in range(NC2):
                if s == 0 and k in (1, 3, 5):
                    dmas = {1: (cw2_t, cw2_d), 3: (cwb_t, cwb_d),
                            5: (idf_t, idf_d)}
                    t_, d_ = dmas[k]
                    nc.sync.dma_start(t_[:], d_)
                if k in (2, 8, 14, 20):
                    q = {2: 0, 8: 1, 14: 2, 20: 3}[k] * 32
                    nc.sync.dma_start(x16_t[:, q:q + 32, :],
                                      x16_d[s, :, q:q + 32, :])
                if s + 1 < SPC and k in (24, 26, 28, 30):
                    # prefetch next sample's x8 bands (SP queue keeps them
                    # behind this sample's loads; ring dep delays the write)
                    if k == 24:
                        xs_next = x8p.tile([C, HP, HP], F8, name="xs")
                        nc.sync.dma_start(xs_next[:, 0:34, :],
                                          x8_d[s + 1, :, 0:34, :])
                    else:
                        r0 = 34 + 32 * ((k - 26) // 2)
                        nc.sync.dma_start(xs_next[:, r0:r0 + 32, :],
                                          x8_d[s + 1, :, r0:r0 + 32, :])
                pch = psc.tile([C, 4, W], F32)
                conv_pairs(pch, xs, cw1_t, k, close=True)
                if k % 8 in (2, 5, 7) and 0.0 <= prelu1 <= 1.0:
                    tmpd = dvp.tile([C, 4, W], F32, name="tmpd")
                    if conv_bias:
                        nc.vector.tensor_scalar(tmpd[:], pch[:], 1.0 / SC,
                                                c1b_t[:], ALU.mult, ALU.add)
                    else:
                        nc.vector.tensor_scalar_mul(tmpd[:], pch[:], 1.0 / SC)
                    # prelu(t) = max(a*t, t) for 0<=a<=1
                    nc.vector.scalar_tensor_tensor(
                        h1_t[:, 4 * k + 1:4 * k + 5, 1:W + 1], tmpd[:],
                        prelu1, tmpd[:], op0=ALU.mult, op1=ALU.max,
                        accum_out=tacc[:, k:k + 1])
                else:
                    nc.scalar.activation(
                        h1_t[:, 4 * k + 1:4 * k + 5, 1:W + 1], pch[:],
                        AF.Prelu, bias=(c1b_t[:] if conv_bias else 0.0),
                        scale=1.0 / SC, alpha=prelu1,
                        accum_out=tacc[:, k:k + 1])

            # ---- marginal sums of h1 -> 9 tap-sums S (DVE, all [C,1])
            _cid = [0]

            def col(shape=(C, 1), dt=F32):
                _cid[0] += 1
                return gsm.tile(list(shape), dt, name=f"g{s}_{_cid[0]}")

            T = col(); nc.vector.tensor_reduce(
                T[:], tacc[:], axis=mybir.AxisListType.X, op=ALU.add)
            rt = col(); nc.vector.tensor_reduce(
                rt[:], h1_t[:, 1, 1:W + 1], axis=mybir.AxisListType.X,
                op=ALU.add)
            rb = col(); nc.vector.tensor_reduce(
                rb[:], h1_t[:, H, 1:W + 1], axis=mybir.AxisListType.X,
                op=ALU.add)
            cl = col(); nc.vector.tensor_reduce(
                cl[:], h1_t[:, 1:H + 1, 1], axis=mybir.AxisListType.X,
                op=ALU.add)
            cr = col(); nc.vector.tensor_reduce(
                cr[:], h1_t[:, 1:H + 1, W], axis=mybir.AxisListType.X,
                op=ALU.add)
            tl = h1_t[:, 1, 1:2]; tr = h1_t[:, 1, W:W + 1]
            bl = h1_t[:, H, 1:2]; br = h1_t[:, H, W:W + 1]
            A0 = col(); nc.vector.tensor_sub(A0[:], T[:], rb[:])
            A2 = col(); nc.vector.tensor_sub(A2[:], T[:], rt[:])
            S = sgp.tile([C, 9], BF16)
            tmp = col(); nc.vector.tensor_sub(tmp[:], A0[:], cr[:])
            nc.vector.tensor_tensor(S[:, 0:1], tmp[:], br, op=ALU.add)
            nc.vector.tensor_sub(S[:, 1:2], T[:], cr[:])
            nc.vector.tensor_scalar_add(S[:, 2:3], A0[:], 0.0)
            nc.vector.tensor_scalar_add(S[:, 3:4], T[:], 0.0)
            tmp = col(); nc.vector.tensor_sub(tmp[:], A0[:], cl[:])
            nc.vector.tensor_tensor(S[:, 4:5], tmp[:], bl, op=ALU.add)
            nc.vector.tensor_sub(S[:, 5:6], T[:], cl[:])
            tmp = col(); nc.vector.tensor_sub(tmp[:], A2[:], cr[:])
            nc.vector.tensor_tensor(S[:, 6:7], tmp[:], tr, op=ALU.add)
            nc.vector.tensor_scalar_add(S[:, 7:8], A2[:], 0.0)
            tmp = col(); nc.vector.tensor_sub(tmp[:], A2[:], cl[:])
            nc.vector.tensor_tensor(S[:, 8:9], tmp[:], tl, op=ALU.add)

            # ---- conv2 with gate chain interleaved; drains emit final y.
            # The first PRE chunks close ungated (h2 -> bf16 tmp, Act) and
            # DVE applies gate+residual+prelu later: no psum sits open
            # waiting for the gate, so PE never stalls on the gate chain.
            pre = PRE if 0.0 <= prelu2 <= 1.0 else 0
            x1c = col(); a_t = col((CH, 1)); gate = col()
            scg = col(); rec = col(); rec64 = col()
            dgt = dgp.tile([C, C], F16)
            h2t = dgp.tile([C, 4 * PRE, W], BF16)
            if conv_bias:
                bsg = col()
            ystage = None
            pend = []

            def flush_one():
                k, pch = pend.pop(0)
                nonlocal ystage
                if k % 4 == 0:
                    ystage = ysp.tile([C, 16, W], F16)
                nc.tensor.matmul(pch[:, 0:4, :], dgt[:],
                                 x16_t[:, 4 * k:4 * k + 4, :],
                                 start=False, stop=True,
                                 skip_group_check=True)
                nc.scalar.activation(
                    ystage[:, (k % 4) * 4:(k % 4) * 4 + 4, :], pch[:],
                    AF.Prelu, bias=(bsg[:] if conv_bias else 0.0),
                    scale=scg[:], alpha=prelu2)
                r0 = 16 * (k // 4)
                dma = nc.sync if (k // 4) % 2 == 0 else nc.gpsimd
                if k // 4 == NC2 // 4 - 1:
                    # last group: store in halves so the final DMA overlaps
                    # the last drains instead of trailing them
                    if k % 4 == 1:
                        dma.dma_start(y_d[s, :, r0:r0 + 8, :],
                                      ystage[:, 0:8, :])
                    elif k % 4 == 3:
                        nc.gpsimd.dma_start(y_d[s, :, r0 + 8:r0 + 16, :],
                                            ystage[:, 8:16, :])
                elif k % 4 == 3:
                    dma.dma_start(y_d[s, :, r0:r0 + 16, :], ystage[:])

            for k in range(NC2):
                if k == STATS_AT:
                    psx = psv.tile([C, 1], F32, name='psv_t')
                    for t in range(9):
                        nc.tensor.matmul(psx[:], cwb_t[:, t, :], S[:, t:t + 1],
                                         start=(t == 0), stop=(t == 8))
                    nc.scalar.copy(x1c[:], psx[:])
                if k == MM1_AT:
                    psa = psv.tile([C, 1], F32, name='psv_t')
                    nc.tensor.matmul(psa[0:CH, :], adp1_t[:, 0:CH], x1c[:],
                                     start=True, stop=True)
                    nc.vector.tensor_scalar(a_t[:], psa[0:CH, :],
                                            adp2_t[:, C:C + 1], 0.0,
                                            ALU.add, ALU.max)
                if k == MM2_AT:
                    psg = psv.tile([C, 1], F32, name='psv_t')
                    nc.tensor.matmul(psg[:], adp2_t[:, 0:C], a_t[:],
                                     start=True, stop=True)
                    nc.scalar.activation(gate[:], psg[:], AF.Sigmoid,
                                         bias=adp1_t[:, CH:CH + 1])
                    nc.vector.tensor_scalar_mul(scg[:], gate[:], 1.0 / SC)
                    if conv_bias:
                        nc.vector.tensor_scalar_mul(bsg[:], gate[:],
                                                    c2b_t[:])
                    nc.vector.reciprocal(rec[:], gate[:])
                    nc.vector.tensor_scalar_mul(rec64[:], rec[:], SC)
                    nc.vector.tensor_scalar_mul(dgt[:], idf_t[:], rec64[:])
                if k == MM2_AT + 2 and pre:
                    # DVE finishes the PRE ungated chunks: gate, residual,
                    # prelu -> y rows 0..4*PRE (its own ystage group(s))
                    for kp in range(0, pre, 4):
                        yst0 = ysp.tile([C, 16, W], F16, name="ystage")
                        for k4 in range(kp, min(kp + 4, pre)):
                            tt = dvp.tile([C, 4, W], F16, name="tt")
                            nc.vector.scalar_tensor_tensor(
                                tt[:], h2t[:, 4 * k4:4 * k4 + 4, :], gate[:],
                                x16_t[:, 4 * k4:4 * k4 + 4, :],
                                op0=ALU.mult, op1=ALU.add)
                            nc.vector.scalar_tensor_tensor(
                                yst0[:, (k4 % 4) * 4:(k4 % 4) * 4 + 4, :],
                                tt[:], prelu2, tt[:],
                                op0=ALU.mult, op1=ALU.max)
                        nc.gpsimd.dma_start(
                            y_d[s, :, 4 * kp:4 * kp + 16, :], yst0[:])
                while len(pend) >= (PSC_BUFS if k <= MM2_AT + 2 else
                                    (2 if k < NC2 - 2 else 1)):
                    flush_one()
                pch = psc.tile([C, 4, W], F32)
                if k < pre:
                    conv_pairs(pch, h1_t, cw2_t, k, close=True)
                    nc.scalar.activation(
                        h2t[:, 4 * k:4 * k + 4, :], pch[:], AF.Identity,
                        bias=(c2b_t[:] if conv_bias else 0.0), scale=1.0 / SC)
                else:
                    conv_pairs(pch, h1_t, cw2_t, k, close=False)
                    pend.append((k, pch))
            while pend:
                flush_one()

    nc.compile()
    return nc


_CACHE = {}


def _get_program(prelu1, prelu2, conv_bias):
    key = (float(prelu1), float(prelu2), bool(conv_bias))
    if key not in _CACHE:
        _CACHE[key] = _build(*key)
    return _CACHE[key]


def _prep(x, intensity, conv1_w, conv1_b, prelu1, conv2_w, conv2_b,
          aW1, ab1, aW2, ab2, prelu2):
    x = np.asarray(x, np.float32)
    idx = np.asarray(intensity).astype(np.int64) - 1
    conv1_w = np.asarray(conv1_w, np.float32)
    conv1_b = np.asarray(conv1_b, np.float32)
    conv2_w = np.asarray(conv2_w, np.float32)
    conv2_b = np.asarray(conv2_b, np.float32)
    aW1 = np.asarray(aW1, np.float32)
    ab1 = np.asarray(ab1, np.float32)
    aW2 = np.asarray(aW2, np.float32)
    ab2 = np.asarray(ab2, np.float32)

    # [Co,Ci,ky,kx] -> [Ci, tap, Co] in DoubleRow pair order + zero pad tap
    def packtaps(w, scale, dt, taps):
        cw = np.zeros((C, len(taps), C), dt)
        for t, tap in enumerate(taps):
            if tap is None:
                continue
            dy, dx = tap
            cw[:, t, :] = (w[:, :, dy, dx].T * scale).astype(dt)
        return cw

    cw1 = packtaps(conv1_w, SC, E4, TAPS)
    cw2 = packtaps(conv2_w, SC, E4, TAPS)
    # stats-path taps: the 9 real taps in (row-major over dy of pairs) order
    cwb = packtaps(conv2_w, 1.0, BF, [t for t in TAPS if t is not None])

    # per-sample expert gather; fold 1/(H*W) into W1^T and W1@conv2_b into b1
    w1t = np.ascontiguousarray(
        (aW1[idx] / float(H * W)).transpose(0, 2, 1))        # [N, C, CH]
    b1g = ab1[idx] + np.einsum('nhc,c->nh', aW1[idx], conv2_b)  # [N, CH]
    w2t = np.ascontiguousarray(aW2[idx].transpose(0, 2, 1))  # [N, CH, C]
    b2g = ab2[idx]                                           # [N, C]
    adp1 = np.concatenate([w1t, b2g[:, :, None]], axis=2)    # [N, C, CH+1]
    adp2 = np.concatenate([w2t, b1g[:, :, None]], axis=2)    # [N, CH, C+1]

    conv_bias = bool(np.any(conv1_b) or np.any(conv2_b))
    nc = _get_program(float(prelu1), float(prelu2), conv_bias)

    x8 = np.zeros((N, C, HP, HP), E4)
    x8[:, :, 1:H + 1, 1:W + 1] = x.astype(E4)
    x16 = x.astype(np.float16)
    idf = np.eye(C, dtype=np.float16)

    in_maps = []
    for i in range(NCORES):
        sl = slice(i * SPC, (i + 1) * SPC)
        m = dict(x8=x8[sl], x16=x16[sl], cw1=cw1, cw2=cw2, cwb=cwb,
                 idf=idf, adp1=adp1[sl], adp2=adp2[sl])
        if conv_bias:
            m["c1b"] = conv1_b[:, None]
            m["c2b"] = conv2_b[:, None]
        in_maps.append(m)
    return nc, in_maps


def kernel(**inputs):
    import time
    from concourse.bass_utils import run_bass_kernel_spmd

    nc, in_maps = _prep(**inputs)
    res = None
    for attempt, pause in enumerate((0, 15, 60, 120)):
        if pause:
            time.sleep(pause)
        try:
            res = run_bass_kernel_spmd(nc, in_maps,
                                       core_ids=list(range(NCORES)))
            break
        except Exception:
            # transient NRT_EXEC_UNIT_UNRECOVERABLE (wedged core); retry
            if attempt == 3:
                raise
    return np.concatenate(
        [np.asarray(r["y"], np.float32) for r in res.results], axis=0)
